# revision 35
# baseline (speedup 1.0000x reference)
"""Trainium2 Bass kernel for nn_ExRestSelfAtten (sparse window attention).

Math reduction (exact):
  reference softmax is over a singleton axis -> attn_w == ones exactly,
  so Wq/Wk are dead and
    out[b,t,o] = sum_s sum_h M[t,s] * relu(x@W1+b1)[b,s,h] * Wvo[h,o] + c[t,o]
  with M[t,s] = 1{|t-s|<=5}, Wvo = Wv@Wo, c folding pos_enc/bo.

Device pipeline per core (4096 batches = 45056 tokens; warmup/body/cooldown
stages of 256/512/128 batches, software-pipelined):
  - x is split on host into fp16 hi/lo planes (x = hi + lo + O(2^-22)); each
    stage's planes are DMA-xbar-transposed into SBUF feature-major as two
    same-mode back-to-back transfers (compute starts on the hi half).  All
    f32 constants and W1 ride inside the same transposed stream (consts as
    u16 half-plane rows), so the whole kernel needs zero xbar transitions.
  - MM1 (fp16, 3 terms: hi*W1hi + hi*W1lo + lo*W1hi) contracts f=128 per
    sequence position s, writing PSUM partition strips via tile_position so
    partitions become (s%4)*32+h; relu+b1 -> R2[g] (g = s//4).
  - MM2 (f32) contracts (s,h) in 3 K-chunks with W2g[(s,h),(t,o)] =
    M[t,s]*Wvo[h,o], accumulating out[(t,o), batch] in PSUM.
  - +c bias, PE-transpose to batch-major, buffered stores at the end.
"""

import os
import sys
import numpy as np
from contextlib import ExitStack

sys.path.insert(0, "/opt/trn_rl_repo")

B, S, D, H, O = 32768, 11, 128, 32, 2
A = 5
WIN = 2 * A + 1
NCORES = 8
NB = B // NCORES              # 4096 batches per core
NTOK = NB * S                 # 45056 tokens per core
TO = S * O                    # 22 = flattened (t, o)

OCTS = 8                      # pipeline stages per core
BOCT = NB // OCTS             # 512 batches per octant
TOKOCT = BOCT * S             # 5632 tokens per octant
KCH = [(0, 4), (4, 8), (8, 11)]   # s-ranges of the 3 (s,h) K-chunks
# (base_batch, n_batches): small warmup stages, 512-batch body, small
# cool-down stages so the PE tail after the last DMA is short
STAGES = ([(0, 256), (256, 256)] + [(512 + 512 * i, 512) for i in range(6)]
          + [(3584, 256), (3840, 128), (3968, 128)])
STORE_GROUPS = [(0, 512, 2), (512, 3584, 4), (3584, 3840, 2),
                (3840, 4096, 1)]

_CACHE = {}


def _build_nc():
    import concourse.bass as bass
    import concourse.tile as tile
    from concourse import bacc, mybir

    f32 = mybir.dt.float32
    f16 = mybir.dt.float16
    Relu = mybir.ActivationFunctionType.Relu
    Ident = mybir.ActivationFunctionType.Identity

    nc = bacc.Bacc()
    # xcat rows: [consts-u16 (192); st0_hi; w1T (64); st0_lo; st1_hi; ...]
    # consts ride the transposed stream as u16 half-planes: row 2c holds the
    # low u16 halves of f32 const column c, row 2c+1 the high halves.
    xcat_ext = nc.dram_tensor("xcat", [2 * NTOK + 256, D], f16,
                              kind="ExternalInput")
    out_ext = nc.dram_tensor("out", [NB, TO], f32, kind="ExternalOutput")

    with tile.TileContext(nc) as tc, ExitStack() as ctx:
        consts = ctx.enter_context(tc.tile_pool(name="consts", bufs=1))
        xtpool = ctx.enter_context(tc.tile_pool(name="xtpool", bufs=4))
        r2pool = ctx.enter_context(tc.tile_pool(name="r2pool", bufs=4))
        osbpool = ctx.enter_context(tc.tile_pool(name="osbpool", bufs=2))
        otpool = ctx.enter_context(tc.tile_pool(name="otpool", bufs=1))
        ps_r2 = ctx.enter_context(tc.tile_pool(name="ps_r2", bufs=2, space="PSUM"))
        ps_o2 = ctx.enter_context(tc.tile_pool(name="ps_o2", bufs=1, space="PSUM"))
        ps_ot = ctx.enter_context(tc.tile_pool(name="ps_ot", bufs=1, space="PSUM"))

        # stage 0 piece 1 carries [consts(192) | hi | w1T(64)] so the whole
        # kernel runs in a single xbar mode with zero transitions.
        T0 = STAGES[0][1] * S
        xt2_0 = xtpool.tile([D, 192 + T0 + 64], f16, tag="xt2a")
        nc.sync.dma_start_transpose(out=xt2_0,
                                    in_=xcat_ext[0:192 + T0 + 64, :])
        cpk_sb = xt2_0[:, 0:180].bitcast(f32)          # (128, 90)
        w2_sb = cpk_sb[:, 0:66]
        b1_sb = cpk_sb[:, 66:67]
        cvec_sb = cpk_sb[:TO, 67:68]
        id22_sb = cpk_sb[:TO, 68:90]
        w1hi_sb = xt2_0[:, 192 + T0:192 + T0 + 32]
        w1lo_sb = xt2_0[:, 192 + T0 + 32:192 + T0 + 64]
        xt2_0b = xtpool.tile([D, T0], f16, tag="xt2b")
        nc.sync.dma_start_transpose(
            out=xt2_0b, in_=xcat_ext[192 + T0 + 64:192 + 2 * T0 + 64, :])

        outbuf = otpool.tile([128, NB // 128, TO], f32)

        def load_stage(st):
            base_b, nb = STAGES[st]
            ntok = nb * S
            tok0 = base_b * 2 * S + 256
            if st == 0:
                xth = xt2_0[:, 192:192 + ntok]
                xtl = xt2_0b[:, :ntok]
            else:
                xt2 = xtpool.tile([D, 2 * TOKOCT], f16, tag="xt2",
                                  name=f"xt2_{st}")[:, :2 * ntok]
                xth = xt2[:, :ntok]
                xtl = xt2[:, ntok:]
                nc.sync.dma_start_transpose(
                    out=xth, in_=xcat_ext[tok0:tok0 + ntok, :])
                nc.sync.dma_start_transpose(
                    out=xtl, in_=xcat_ext[tok0 + ntok:tok0 + 2 * ntok, :])
            return xth, xtl

        def mm1_relu(st, xth, xtl):
            base_b, nb = STAGES[st]
            nbl = nb // 128
            # token t = (nbl*11)p + 11bl + s ; column n = bl*128+p = batch
            xthv = xth.rearrange("f (p bl s) -> f s bl p", p=128, bl=nbl, s=S)
            xtlv = xtl.rearrange("f (p bl s) -> f s bl p", p=128, bl=nbl, s=S)
            r2ps = [ps_r2.tile([128, BOCT], f32, name=f"r2ps{g}_{st}",
                               tag=f"r2ps{g}")[:, :nb]
                    for g in range(3)]
            # grouped by stationary operand to minimize weight reloads
            for wsb, xv, st_, sp in [(w1hi_sb, xthv, True, False),
                                     (w1hi_sb, xtlv, False, False),
                                     (w1lo_sb, xthv, False, True)]:
                for s in range(S):
                    g, sm = s // 4, s % 4
                    nc.tensor.matmul(
                        r2ps[g][32 * sm:32 * sm + 32, :], wsb, xv[:, s],
                        start=st_, stop=sp, tile_position=(0, 32 * sm))
            r2 = [r2pool.tile([128, BOCT], f32, name=f"r2{g}_{st}",
                              tag=f"r2{g}")[:, :nb]
                  for g in range(3)]
            for g, (s0, s1) in enumerate(KCH):
                np_ = 32 * (s1 - s0)
                nc.scalar.activation(
                    out=r2[g][:np_, :], in_=r2ps[g][:np_, :], func=Relu,
                    bias=b1_sb[:np_], scale=1.0)
            return r2

        def mm2_out(st, r2):
            base_b, nb = STAGES[st]
            nbl = nb // 128
            o2 = ps_o2.tile([TO, BOCT], f32, name=f"o2_{st}", tag="o2")[:, :nb]
            for g, (s0, s1) in enumerate(KCH):
                np_ = 32 * (s1 - s0)
                nc.tensor.matmul(
                    o2, w2_sb[:np_, g * TO:(g + 1) * TO], r2[g][:np_, :],
                    start=(g == 0), stop=(g == 2))
            osb = osbpool.tile([TO, BOCT], f32, name=f"osb_{st}",
                               tag="osb")[:, :nb]
            nc.scalar.activation(out=osb, in_=o2, func=Ident, bias=cvec_sb,
                                 scale=1.0)
            oTp = ps_ot.tile([128, 4, TO], f32, name=f"oTp_{st}",
                             tag="oTp")[:, :nbl]
            for blk in range(nbl):
                nc.tensor.transpose(
                    oTp[:, blk, :], osb[:, blk * 128:(blk + 1) * 128], id22_sb)
            nc.scalar.copy(outbuf[:, base_b // 128:base_b // 128 + nbl], oTp)

        # software pipeline: MM1(n+1) is issued before MM2(n) so the PE
        # always has independent work while ACT drains PSUM.
        xth, xtl = load_stage(0)
        r2_prev = mm1_relu(0, xth, xtl)
        for st in range(1, len(STAGES)):
            xth, xtl = load_stage(st)
            r2_next = mm1_relu(st, xth, xtl)
            mm2_out(st - 1, r2_prev)
            r2_prev = r2_next
        mm2_out(len(STAGES) - 1, r2_prev)

        # stores at the end, one per uniform stage group
        for b0, b1_, nbl in STORE_GROUPS:
            ncs = (b1_ - b0) // (nbl * 128)
            dst = out_ext[b0:b1_, :].rearrange(
                "(cs p bl) to -> p cs (bl to)", cs=ncs, p=128, bl=nbl)
            srcv = outbuf[:, b0 // 128:b1_ // 128].rearrange(
                "p (cs bl) to -> p cs (bl to)", cs=ncs, bl=nbl)
            nc.sync.dma_start(out=dst, in_=srcv)

    nc.finalize()
    return nc


def _get_nc():
    if "nc" not in _CACHE:
        _CACHE["nc"] = _build_nc()
    return _CACHE["nc"]


def _fold_weights(W1, b1, Wv, pos_enc, Wo, bo):
    Wvo = Wv.astype(np.float64) @ Wo.astype(np.float64)          # (32, 2)
    t_idx = np.arange(S)
    M = (np.abs(t_idx[:, None] - t_idx[None, :]) <= A).astype(np.float64)  # (t, s)
    # W2[(s-s0)*32+h, g*22 + t*2+o] = M[t, s] * Wvo[h, o]
    w2 = np.zeros((128, 3 * TO), np.float64)
    for g, (s0, s1) in enumerate(KCH):
        blk = np.einsum("st,ho->shto", M.T[s0:s1], Wvo).reshape(
            (s1 - s0) * H, TO)
        w2[:(s1 - s0) * H, g * TO:(g + 1) * TO] = blk
    pos = pos_enc.reshape(S, H).astype(np.float64)
    cvec = (M @ pos) @ Wvo + bo.reshape(1, O).astype(np.float64)  # (t, o)
    b1r = np.tile(b1.reshape(1, H), (4, 1)).reshape(128, 1)
    w1hi = W1.astype(np.float16)
    w1lo = (W1.astype(np.float64) - w1hi.astype(np.float64)).astype(np.float16)
    return (w2.astype(np.float32), cvec.reshape(TO, 1).astype(np.float32),
            b1r.astype(np.float32), w1hi, w1lo)


def kernel(x, W1, b1, Wq, Wk, Wv, pos_enc, Wo, bo):
    from concourse.bass_utils import run_bass_kernel_spmd

    x = np.asarray(x, dtype=np.float32)
    assert x.shape == (B, S, D), x.shape
    w2, cvec, b1r, w1hi, w1lo = _fold_weights(
        np.asarray(W1, np.float32), np.asarray(b1, np.float32),
        np.asarray(Wv, np.float32), np.asarray(pos_enc, np.float32),
        np.asarray(Wo, np.float32), np.asarray(bo, np.float32))
    cpk = np.zeros((128, 90), np.float32)
    cpk[:, 0:66] = w2
    cpk[:, 66:67] = b1r
    cpk[:TO, 67:68] = cvec
    cpk[:TO, 68:90] = np.eye(TO, dtype=np.float32)

    xf = x.reshape(B * S, D)
    xhi = xf.astype(np.float16)
    xlo = (xf - xhi.astype(np.float32)).astype(np.float16)
    # per stage: [hi_stage; lo_stage] row blocks, with w1T (64 rows)
    # inserted after stage 0's hi block
    xhi_c = xhi.reshape(NCORES, NTOK, D)
    xlo_c = xlo.reshape(NCORES, NTOK, D)
    w1t = np.concatenate([w1hi.T, w1lo.T], axis=0)       # (64, 128) f16
    # consts as u16 half-plane rows (192 rows incl pad), see kernel comment
    cw = cpk.view(np.uint16).reshape(128, 90, 2).transpose(1, 2, 0)
    cw = cw.reshape(180, 128)
    cw = np.concatenate([cw, np.zeros((12, 128), np.uint16)], axis=0)
    cw16 = cw.view(np.float16)
    parts = [np.broadcast_to(cw16, (NCORES, 192, D))]
    for st, (base_b, nb) in enumerate(STAGES):
        t0, t1 = base_b * S, (base_b + nb) * S
        parts.append(xhi_c[:, t0:t1])
        if st == 0:
            parts.append(np.broadcast_to(w1t, (NCORES, 64, D)))
        parts.append(xlo_c[:, t0:t1])
    xcat = np.concatenate(parts, axis=1)

    nc = _get_nc()
    in_maps = []
    for i in range(NCORES):
        in_maps.append({"xcat": np.ascontiguousarray(xcat[i])})

    res = run_bass_kernel_spmd(nc, in_maps, list(range(NCORES)), trace=False)
    _CACHE["last_result"] = res

    out = np.concatenate([r["out"] for r in res.results], axis=0)
    out = out.reshape(B, S, O)
    attn_w = np.ones((B, S, 1, WIN), dtype=np.float32)
    return out, attn_w
